# revision 35
# baseline (speedup 1.0000x reference)
"""MixtureLinear Trainium2 kernel.

Computes, for B=256, IN=1024, OUT=1024, RANK=16:
    out[b,o] = sum_i input[b,i] * sum_r weight[o,i,r] * coef[b,r]
             + sum_r bias[o,r] * coef[b,r]

Strategy (8 NeuronCores, tensor-parallel on OUT):
  - Core c owns OUT rows [128c, 128c+128). It reads only its weight shard
    (1/8 of the 64MB weight tensor), input/coef replicated.
  - Stage 1 (PE): proj[b,(o,r)] = inputT.T @ W2 where W2[i, o*16+r] =
    weight[o,i,r]; K=IN accumulated over 8 psum matmuls per 512-column
    chunk (one psum bank, 32 o's x 16 r's per chunk).
  - Stage 2 (DVE): out[b,o] = sum_r proj[b,(o,r)] * coef[b,r] via a
    broadcast-AP multiply + strided reduce over the innermost rank axis.
  - Bias: one tiny K=16 matmul per b-chunk: coefT.T @ biasT -> psum,
    added in the final DVE add before the output DMA.

Matmul dtype is selectable via MIXL_DT (float16 default; bfloat16 /
float32r / float32 supported). Host pre-casts and pre-transposes shards;
stage-2 and all accumulation stay fp32.
"""

import os
import sys
from contextlib import ExitStack

sys.path.insert(0, "/opt/trn_rl_repo")

import numpy as np
import ml_dtypes

import concourse.bass as bass
import concourse.tile as tile
from concourse import bacc, mybir
from concourse.bass_utils import run_bass_kernel_spmd

B, IN, OUT, RANK = 256, 1024, 1024, 16
NCORES = 8
OUTL = OUT // NCORES        # 128 out rows per core
P = 128                     # partitions
NB = B // P                 # 2 batch chunks
NK = IN // P                # 8 contraction chunks
CH = 512                    # psum chunk: one fp32 bank
NCH = OUTL * RANK // CH     # 4 column chunks per core
OCH = CH // RANK            # 32 o's per chunk

DT_NAME = os.environ.get("MIXL_DT", "float16")
IMPL = os.environ.get("MIXL_IMPL", "raw")

_DT_MAP = {
    "float16": (mybir.dt.float16, np.float16),
    "bfloat16": (mybir.dt.bfloat16, ml_dtypes.bfloat16),
    "float32r": (mybir.dt.float32r, np.float32),
    "float32": (mybir.dt.float32, np.float32),
}


def build_nc_raw(dt_name=DT_NAME):
    """Raw-Bass (manual Block + semaphores) implementation.

    The Tile framework adds ~7us of preamble (serial all-engine barrier
    chain, pool memsets) and ~8us of teardown (drain + EVSEM butterfly +
    sem recycling) — measured 15us of a 38us kernel. This version keeps
    only the work: two HWDGE trigger streams (SP: weights+outputs,
    ACT: inputT), SWDGE for the tiny tensors, PE matmuls, DVE stage-2,
    and a gpsimd tail that waits for the output DMA and resets the
    semaphores so the NEFF stays re-executable.
    """
    dt, _ = _DT_MAP[dt_name]
    f32 = mybir.dt.float32
    nc = bacc.Bacc("TRN2", target_bir_lowering=False, debug=False)

    xT = nc.declare_dram_parameter("xT", [IN, B], dt, isOutput=False)
    w2 = nc.declare_dram_parameter("w2", [NCH, P, NK * CH], dt, isOutput=False)
    coef = nc.declare_dram_parameter("coef", [B, RANK], f32, isOutput=False)
    coefT = nc.declare_dram_parameter("coefT", [RANK, B], dt, isOutput=False)
    biasT = nc.declare_dram_parameter("biasT", [RANK, OUTL], dt, isOutput=False)
    out = nc.declare_dram_parameter("out", [B, OUTL], f32, isOutput=True)

    w2v = w2.rearrange("n p (k c) -> n p k c", c=CH)
    xTv = xT.rearrange("(k p) b -> p k b", p=P)
    coefv = coef.rearrange("(nb p) r -> p nb r", p=P)

    with ExitStack() as ctx:
        sb = lambda shape, d, name: ctx.enter_context(
            nc.sbuf_tensor(name, shape, d))
        xT_t = sb([P, NK, B], dt, "xT_t")
        wts = [sb([P, NK, CH], dt, f"wt{n}") for n in range(NCH)]
        coef_t = sb([P, NB, RANK], f32, "coef_t")
        coefT_t = sb([RANK, B], dt, "coefT_t")
        biasT_t = sb([RANK, OUTL], dt, "biasT_t")
        tmps = [sb([P, OCH, RANK], f32, f"tmp{i}") for i in range(2)]
        out_sb = [sb([P, OUTL], f32, f"osum{b}") for b in range(NB)]
        outf = [sb([P, OUTL], f32, f"outf{b}") for b in range(NB)]
        pss = [ctx.enter_context(nc.psum_tensor(f"ps{g}", [P, CH], f32))
               for g in range(8)]

        # One semaphore per DMA: +16 increments from different transfers
        # interleave (per-SDMA-engine +1s), so aggregate thresholds on a
        # shared sem do not prove any single transfer completed.
        s_wc = [ctx.enter_context(nc.semaphore(f"s_w{i}"))
                for i in range(NCH + 1)]                  # w0a, w0b, w1, w2, w3
        s_x1 = ctx.enter_context(nc.semaphore("s_x1"))    # inputT low half
        s_x2 = ctx.enter_context(nc.semaphore("s_x2"))    # inputT high half
        s_gc = ctx.enter_context(nc.semaphore("s_gc"))    # coefT
        s_gb = ctx.enter_context(nc.semaphore("s_gb"))    # biasT
        s_gf = ctx.enter_context(nc.semaphore("s_gf"))    # coef (fp32)
        s_pe = ctx.enter_context(nc.semaphore("s_pe"))    # psum groups done
        s_dvm = ctx.enter_context(nc.semaphore("s_dvm"))  # psum mults done
        s_red = ctx.enter_context(nc.semaphore("s_red"))  # reduces done
        s_dve = ctx.enter_context(nc.semaphore("s_dve"))  # outf ready
        s_out = ctx.enter_context(nc.semaphore("s_out"))  # output DMA done
        sems = s_wc + [s_x1, s_x2, s_gc, s_gb, s_gf,
                       s_pe, s_dvm, s_red, s_dve, s_out]

        with nc.Block() as block:

            @block.sync
            def _(sync):
                # Weight stream: n=0 split so the first matmuls start after
                # only 256KB. One HWDGE ring -> transfers run FIFO.
                sync.dma_start(wts[0][:, 0:2, :], w2v[0][:, 0:2, :]).then_inc(
                    s_wc[0], 16)
                sync.dma_start(wts[0][:, 2:, :], w2v[0][:, 2:, :]).then_inc(
                    s_wc[1], 16)
                for n in range(1, NCH):
                    sync.dma_start(wts[n][:], w2v[n]).then_inc(s_wc[n + 1], 16)
                for b in range(NB):
                    sync.wait_ge(s_dve, b + 1)
                    sync.dma_start(out[b * P:(b + 1) * P, :], outf[b][:]).then_inc(
                        s_out, 16)

            @block.scalar
            def _(scalar):
                scalar.dma_start(xT_t[:, 0:NK // 2, :],
                                 xTv[:, 0:NK // 2, :]).then_inc(s_x1, 16)
                scalar.dma_start(xT_t[:, NK // 2:, :],
                                 xTv[:, NK // 2:, :]).then_inc(s_x2, 16)

            @block.gpsimd
            def _(gpsimd):
                gpsimd.dma_start(coef_t[:], coefv).then_inc(s_gf, 16)
                gpsimd.dma_start(coefT_t[:], coefT[:]).then_inc(s_gc, 16)
                gpsimd.dma_start(biasT_t[:], biasT[:]).then_inc(s_gb, 16)
                # Terminal waiter: holds the Pool stream (and thus the
                # Block-end drain+barrier) until outputs are in DRAM.
                gpsimd.wait_ge(s_out, 32)

            @block.tensor
            def _(pe):
                for n in range(NCH):
                    for k in range(NK):
                        if n == 0 and k == 0:
                            pe.wait_ge(s_x1, 16)
                            pe.wait_ge(s_wc[0], 16)
                        if n == 0 and k == 2:
                            pe.wait_ge(s_wc[1], 16)
                        if n == 0 and k == NK // 2:
                            pe.wait_ge(s_x2, 16)
                        if n >= 1 and k == 0:
                            pe.wait_ge(s_wc[n + 1], 16)
                        for b in range(NB):
                            mm = nc.tensor.matmul(
                                pss[2 * n + b][:],
                                lhsT=xT_t[:, k, b * P:(b + 1) * P],
                                rhs=wts[n][:, k, :],
                                start=(k == 0),
                                stop=(k == NK - 1),
                            )
                            if k == NK - 1:
                                mm.then_inc(s_pe, 1)
                # Bias matmuls at the end, reusing banks 0/1 (freed once
                # their stage-2 multiplies are done).
                pe.wait_ge(s_gc, 16)
                pe.wait_ge(s_gb, 16)
                pe.wait_ge(s_dvm, 2)
                for b in range(NB):
                    nc.tensor.matmul(
                        pss[b][:, 0:OUTL],
                        lhsT=coefT_t[:, b * P:(b + 1) * P],
                        rhs=biasT_t[:],
                        start=True, stop=True,
                    ).then_inc(s_pe, 1)

            @block.vector
            def _(vector):
                vector.wait_ge(s_gf, 16)
                g = 0
                for n in range(NCH):
                    for b in range(NB):
                        g += 1
                        vector.wait_ge(s_pe, g)
                        coef_b = coef_t[:, b, :].rearrange(
                            "p (one r) -> p one r", one=1)
                        tmp = tmps[g % 2]
                        nc.vector.tensor_mul(
                            tmp[:],
                            pss[2 * n + b][:].rearrange("p (o r) -> p o r", r=RANK),
                            coef_b.to_broadcast((P, OCH, RANK)),
                        ).then_inc(s_dvm, 1)
                        vector.wait_ge(s_dvm, g)
                        nc.vector.tensor_reduce(
                            out_sb[b][:, n * OCH:(n + 1) * OCH],
                            tmp[:],
                            axis=mybir.AxisListType.X,
                            op=mybir.AluOpType.add,
                        ).then_inc(s_red, 1)
                for b in range(NB):
                    vector.wait_ge(s_pe, NB * NCH + b + 1)
                    # all four reduces of this b-chunk (groups b+1, b+3, ...)
                    vector.wait_ge(s_red, NB * NCH - NB + b + 1)
                    nc.vector.tensor_add(
                        outf[b][:], out_sb[b][:], pss[b][:, 0:OUTL]
                    ).then_inc(s_dve, 1)

        # After the Block-end drain + all-engine barrier: reset semaphores
        # so a reloaded NEFF can execute again.
        nums = sorted(s.num for s in sems)
        rng = range(nums[0], nums[-1] + 1)
        nc.gpsimd.dma_reset(rng)
        nc.gpsimd.sem_clear(rng)

    nc.compile()
    return nc


def build_nc_tile(dt_name=DT_NAME):
    dt, _ = _DT_MAP[dt_name]
    f32 = mybir.dt.float32
    # Bacc (not raw Bass): its compile() runs generate_event_semaphores,
    # which splits multi-wait sync_info into EventSemaphore prefixes —
    # walrus accepts at most one wait per regular instruction.
    nc = bacc.Bacc("TRN2", target_bir_lowering=False, debug=False)

    xT = nc.declare_dram_parameter("xT", [IN, B], dt, isOutput=False)
    # w2[n, p, k*CH+c] = W2[k*128+p, n*CH+c]: pre-swizzled on host so each
    # SBUF partition's data is one contiguous 8KB run in DRAM (full-rate DMA).
    w2 = nc.declare_dram_parameter("w2", [NCH, P, NK * CH], dt, isOutput=False)
    coef = nc.declare_dram_parameter("coef", [B, RANK], f32, isOutput=False)
    coefT = nc.declare_dram_parameter("coefT", [RANK, B], dt, isOutput=False)
    biasT = nc.declare_dram_parameter("biasT", [RANK, OUTL], dt, isOutput=False)
    out = nc.declare_dram_parameter("out", [B, OUTL], f32, isOutput=True)

    with tile.TileContext(nc) as tc, ExitStack() as ctx:
        cpool = ctx.enter_context(tc.tile_pool(name="const", bufs=1))
        wpool = ctx.enter_context(tc.tile_pool(name="w", bufs=NCH))
        ppool = ctx.enter_context(tc.tile_pool(name="proj", bufs=6, space="PSUM"))
        bpool = ctx.enter_context(tc.tile_pool(name="biasps", bufs=2, space="PSUM"))
        spool = ctx.enter_context(tc.tile_pool(name="stage2", bufs=4))
        opool = ctx.enter_context(tc.tile_pool(name="outp", bufs=2))

        # Weight tiles for every n-chunk (issued first; n=0 split so the
        # first matmuls can start after only 256KB has landed).
        wts = [wpool.tile([P, NK, CH], dt, tag="w", name=f"wt{n}")
               for n in range(NCH)]
        w2v = w2.rearrange("n p (k c) -> n p k c", c=CH)
        nc.sync.dma_start(wts[0][:, 0:2, :], w2v[0][:, 0:2, :])
        # Full inputT, split in halves (first matmuls need only low k).
        xT_t = cpool.tile([P, NK, B], dt, tag="xT")
        xTv = xT.rearrange("(k p) b -> p k b", p=P)
        nc.sync.dma_start(xT_t[:, 0:NK // 2, :], xTv[:, 0:NK // 2, :])
        nc.sync.dma_start(wts[0][:, 2:NK, :], w2v[0][:, 2:NK, :])
        nc.sync.dma_start(xT_t[:, NK // 2:, :], xTv[:, NK // 2:, :])
        for n in range(1, NCH):
            nc.sync.dma_start(wts[n][:], w2[n].rearrange("p (k c) -> p k c", c=CH))
        coef_t = cpool.tile([P, NB, RANK], f32, tag="coef")
        nc.sync.dma_start(coef_t[:], coef.rearrange("(nb p) r -> p nb r", p=P))
        coefT_t = cpool.tile([RANK, B], dt, tag="coefT")
        nc.sync.dma_start(coefT_t[:], coefT[:])
        biasT_t = cpool.tile([RANK, OUTL], dt, tag="biasT")
        nc.sync.dma_start(biasT_t[:], biasT[:])

        # Bias term: out_bias[b,o] = sum_r coef[b,r] * bias[o,r]
        bias_ps = []
        for b in range(NB):
            bp = bpool.tile([P, OUTL], f32, tag="bias")
            nc.tensor.matmul(
                bp[:], lhsT=coefT_t[:, b * P:(b + 1) * P], rhs=biasT_t[:],
                start=True, stop=True,
            )
            bias_ps.append(bp)

        out_sb = [
            opool.tile([P, OUTL], f32, tag="osum", name=f"osum{b}")
            for b in range(NB)
        ]

        for n in range(NCH):
            pss = [
                ppool.tile([P, CH], f32, tag="proj", name=f"proj{n}_{b}")
                for b in range(NB)
            ]
            wt = wts[n]
            for k in range(NK):
                for b in range(NB):
                    nc.tensor.matmul(
                        pss[b][:],
                        lhsT=xT_t[:, k, b * P:(b + 1) * P],
                        rhs=wt[:, k, :],
                        start=(k == 0),
                        stop=(k == NK - 1),
                    )
            # Rank contraction: multiply by per-(b,r) coef, reduce over r.
            for b in range(NB):
                tmp = spool.tile([P, CH], f32, tag="tmp")
                coef_b = coef_t[:, b, :].rearrange("p (one r) -> p one r", one=1)
                nc.vector.tensor_mul(
                    tmp[:].rearrange("p (o r) -> p o r", r=RANK),
                    pss[b][:].rearrange("p (o r) -> p o r", r=RANK),
                    coef_b.to_broadcast((P, OCH, RANK)),
                )
                nc.vector.tensor_reduce(
                    out_sb[b][:, n * OCH:(n + 1) * OCH],
                    tmp[:].rearrange("p (o r) -> p o r", r=RANK),
                    axis=mybir.AxisListType.X,
                    op=mybir.AluOpType.add,
                )

        for b in range(NB):
            outf = opool.tile([P, OUTL], f32, tag="outf")
            nc.vector.tensor_add(outf[:], out_sb[b][:], bias_ps[b][:])
            nc.sync.dma_start(out[b * P:(b + 1) * P, :], outf[:])

    nc.compile()
    return nc


def prepare_in_maps(input, coef, weight, bias, dt_name=DT_NAME):
    _, npdt = _DT_MAP[dt_name]
    xT = np.ascontiguousarray(input.T).astype(npdt)          # (IN, B)
    coefT = np.ascontiguousarray(coef.T).astype(npdt)        # (RANK, B)
    coef32 = np.ascontiguousarray(coef.astype(np.float32))   # (B, RANK)
    in_maps = []
    for c in range(NCORES):
        wsh = weight[c * OUTL:(c + 1) * OUTL]                # (OUTL, IN, RANK)
        # W2[i, o*RANK+r] = wsh[o, i, r]; n-major 512-col chunks; then swizzle
        # (n, i=k*128+p, c) -> (n, p, k, c) so each partition reads one
        # contiguous 8KB run per n-chunk DMA.
        w2 = wsh.transpose(1, 0, 2).reshape(IN, OUTL * RANK)
        w2 = w2.reshape(NK, P, NCH, CH).transpose(2, 1, 0, 3)
        w2 = np.ascontiguousarray(w2.reshape(NCH, P, NK * CH)).astype(npdt)
        biasT = np.ascontiguousarray(
            bias[c * OUTL:(c + 1) * OUTL].T
        ).astype(npdt)                                       # (RANK, OUTL)
        in_maps.append({
            "xT": xT, "w2": w2, "coef": coef32,
            "coefT": coefT, "biasT": biasT,
        })
    return in_maps


_NC_CACHE = {}


def _ensure_ntff_hook():
    """The agent image's antenv lacks axon_hooks; inject it and register
    the ctypes NTFF profile hook so trace=True works under axon."""
    import types
    import antenv
    try:
        from antenv import axon_hooks  # noqa: F401
        return
    except ImportError:
        pass
    mod = types.ModuleType("antenv.axon_hooks")
    _state = {"hook": None}
    mod.set_axon_ntff_profile_hook = lambda h: _state.__setitem__("hook", h)
    mod.get_axon_ntff_profile_hook = lambda: _state["hook"]
    sys.modules["antenv.axon_hooks"] = mod
    antenv.axon_hooks = mod
    try:
        from trn_agent_boot.trn_boot import _ntff_profile_via_ctypes
        mod.set_axon_ntff_profile_hook(
            _ntff_profile_via_ctypes("/opt/axon/libaxon_pjrt.so")
        )
    except Exception:
        pass


def build_nc(dt_name=DT_NAME, impl=None):
    impl = impl or IMPL
    if impl == "raw":
        return build_nc_raw(dt_name)
    return build_nc_tile(dt_name)


def run(inputs, trace=False, dt_name=DT_NAME, impl=None, **kwargs):
    if trace:
        _ensure_ntff_hook()
    impl = impl or IMPL
    key = (dt_name, impl)
    if key not in _NC_CACHE:
        _NC_CACHE[key] = build_nc(dt_name, impl)
    nc = _NC_CACHE[key]
    in_maps = prepare_in_maps(
        np.asarray(inputs["input"], dtype=np.float32),
        np.asarray(inputs["coef"], dtype=np.float32),
        np.asarray(inputs["weight"], dtype=np.float32),
        np.asarray(inputs["bias"], dtype=np.float32),
        dt_name,
    )
    br = run_bass_kernel_spmd(
        nc, in_maps, list(range(NCORES)), trace=trace, **kwargs
    )
    full = np.concatenate(
        [br.results[c]["out"] for c in range(NCORES)], axis=1
    ).astype(np.float32)
    return full, br


def kernel(**inputs):
    full, _ = run(inputs)
    return full


# revision 36
# speedup vs baseline: 1.2618x; 1.2618x over previous
"""MixtureLinear Trainium2 kernel.

Computes, for B=256, IN=1024, OUT=1024, RANK=16:
    out[b,o] = sum_i input[b,i] * sum_r weight[o,i,r] * coef[b,r]
             + sum_r bias[o,r] * coef[b,r]

Strategy (8 NeuronCores, tensor-parallel on OUT):
  - Core c owns OUT rows [128c, 128c+128). It reads only its weight shard
    (1/8 of the 64MB weight tensor), input/coef replicated.
  - Stage 1 (PE): proj[b,(o,r)] = inputT.T @ W2 where W2[i, o*16+r] =
    weight[o,i,r]; K=IN accumulated over 8 psum matmuls per 512-column
    chunk (one psum bank, 32 o's x 16 r's per chunk).
  - Stage 2 (DVE): out[b,o] = sum_r proj[b,(o,r)] * coef[b,r] via a
    broadcast-AP multiply + strided reduce over the innermost rank axis.
  - Bias: one tiny K=16 matmul per b-chunk: coefT.T @ biasT -> psum,
    added in the final DVE add before the output DMA.

Matmul dtype is selectable via MIXL_DT (float16 default; bfloat16 /
float32r / float32 supported). Host pre-casts and pre-transposes shards;
stage-2 and all accumulation stay fp32.
"""

import os
import sys
from contextlib import ExitStack

sys.path.insert(0, "/opt/trn_rl_repo")

import numpy as np
import ml_dtypes

import concourse.bass as bass
import concourse.tile as tile
from concourse import bacc, mybir
from concourse.bass_utils import run_bass_kernel_spmd

B, IN, OUT, RANK = 256, 1024, 1024, 16
NCORES = 8
OUTL = OUT // NCORES        # 128 out rows per core
P = 128                     # partitions
NB = B // P                 # 2 batch chunks
NK = IN // P                # 8 contraction chunks
CH = 512                    # psum chunk: one fp32 bank
NCH = OUTL * RANK // CH     # 4 column chunks per core
OCH = CH // RANK            # 32 o's per chunk

DT_NAME = os.environ.get("MIXL_DT", "float16")
IMPL = os.environ.get("MIXL_IMPL", "raw")

_DT_MAP = {
    "float16": (mybir.dt.float16, np.float16),
    "bfloat16": (mybir.dt.bfloat16, ml_dtypes.bfloat16),
    "float32r": (mybir.dt.float32r, np.float32),
    "float32": (mybir.dt.float32, np.float32),
}


def build_nc_raw(dt_name=DT_NAME):
    """Raw-Bass (manual Block + semaphores) implementation.

    The Tile framework adds ~7us of preamble (serial all-engine barrier
    chain, pool memsets) and ~8us of teardown (drain + EVSEM butterfly +
    sem recycling) — measured 15us of a 38us kernel. This version keeps
    only the work: two HWDGE trigger streams (SP: weights+outputs,
    ACT: inputT), SWDGE for the tiny tensors, PE matmuls, DVE stage-2,
    and a gpsimd tail that waits for the output DMA and resets the
    semaphores so the NEFF stays re-executable.
    """
    dt, _ = _DT_MAP[dt_name]
    f32 = mybir.dt.float32
    nc = bacc.Bacc("TRN2", target_bir_lowering=False, debug=False)

    xT = nc.declare_dram_parameter("xT", [IN, B], dt, isOutput=False)
    w2 = nc.declare_dram_parameter("w2", [NCH, P, NK * CH], dt, isOutput=False)
    coef = nc.declare_dram_parameter("coef", [B, RANK], f32, isOutput=False)
    coefT = nc.declare_dram_parameter("coefT", [RANK, B], dt, isOutput=False)
    biasT = nc.declare_dram_parameter("biasT", [RANK, OUTL], dt, isOutput=False)
    out = nc.declare_dram_parameter("out", [B, OUTL], f32, isOutput=True)

    w2v = w2.rearrange("n p (k c) -> n p k c", c=CH)
    xTv = xT.rearrange("(k p) b -> p k b", p=P)
    coefv = coef.rearrange("(nb p) r -> p nb r", p=P)

    with ExitStack() as ctx:
        sb = lambda shape, d, name: ctx.enter_context(
            nc.sbuf_tensor(name, shape, d))
        xT_t = sb([P, NK, B], dt, "xT_t")
        wts = [sb([P, NK, CH], dt, f"wt{n}") for n in range(NCH)]
        coef_t = sb([P, NB, RANK], f32, "coef_t")
        coefT_t = sb([RANK, B], dt, "coefT_t")
        biasT_t = sb([RANK, OUTL], dt, "biasT_t")
        tmps = [sb([P, OCH, RANK], f32, f"tmp{i}") for i in range(2)]
        out_sb = [sb([P, OUTL], f32, f"osum{b}") for b in range(NB)]
        outf = [sb([P, OUTL], f32, f"outf{b}") for b in range(NB)]
        pss = [ctx.enter_context(nc.psum_tensor(f"ps{g}", [P, CH], f32))
               for g in range(8)]

        # One semaphore per DMA: +16 increments from different transfers
        # interleave (per-SDMA-engine +1s), so aggregate thresholds on a
        # shared sem do not prove any single transfer completed.
        s_wc = [ctx.enter_context(nc.semaphore(f"s_w{i}"))
                for i in range(NCH + 1)]                  # w0a, w0b, w1, w2, w3
        s_x1 = ctx.enter_context(nc.semaphore("s_x1"))    # inputT low half
        s_x2 = ctx.enter_context(nc.semaphore("s_x2"))    # inputT high half
        s_gc = ctx.enter_context(nc.semaphore("s_gc"))    # coefT
        s_gb = ctx.enter_context(nc.semaphore("s_gb"))    # biasT
        s_gf = ctx.enter_context(nc.semaphore("s_gf"))    # coef (fp32)
        s_pe = ctx.enter_context(nc.semaphore("s_pe"))    # psum groups done
        s_dvm = ctx.enter_context(nc.semaphore("s_dvm"))  # psum mults done
        s_red = ctx.enter_context(nc.semaphore("s_red"))  # reduces done
        s_dve = ctx.enter_context(nc.semaphore("s_dve"))  # outf ready
        s_out = ctx.enter_context(nc.semaphore("s_out"))  # output DMA done
        sems = s_wc + [s_x1, s_x2, s_gc, s_gb, s_gf,
                       s_pe, s_dvm, s_red, s_dve, s_out]

        with nc.Block() as block:

            @block.sync
            def _(sync):
                # All big loads on ONE ring, paced so at most ~2 transfers
                # are in flight: the SDMA engines round-robin across every
                # queued transfer at packet granularity, so queueing all of
                # them up front makes the first-needed chunk finish last.
                sync.dma_start(xT_t[:, 0:NK // 2, :],
                               xTv[:, 0:NK // 2, :]).then_inc(s_x1, 16)
                sync.dma_start(wts[0][:, 0:2, :], w2v[0][:, 0:2, :]).then_inc(
                    s_wc[0], 16)
                sync.wait_ge(s_x1, 16)
                sync.dma_start(xT_t[:, NK // 2:, :],
                               xTv[:, NK // 2:, :]).then_inc(s_x2, 16)
                sync.wait_ge(s_wc[0], 16)
                sync.dma_start(wts[0][:, 2:, :], w2v[0][:, 2:, :]).then_inc(
                    s_wc[1], 16)
                sync.wait_ge(s_x2, 16)
                sync.dma_start(wts[1][:], w2v[1]).then_inc(s_wc[2], 16)
                sync.wait_ge(s_wc[1], 16)
                sync.dma_start(wts[2][:], w2v[2]).then_inc(s_wc[3], 16)
                sync.wait_ge(s_wc[2], 16)
                sync.dma_start(wts[3][:], w2v[3]).then_inc(s_wc[4], 16)

            @block.scalar
            def _(scalar):
                # Output DMAs on the (otherwise idle) ACT ring.
                for b in range(NB):
                    scalar.wait_ge(s_dve, b + 1)
                    scalar.dma_start(out[b * P:(b + 1) * P, :],
                                     outf[b][:]).then_inc(s_out, 16)

            @block.gpsimd
            def _(gpsimd):
                gpsimd.dma_start(coef_t[:], coefv).then_inc(s_gf, 16)
                gpsimd.dma_start(coefT_t[:], coefT[:]).then_inc(s_gc, 16)
                gpsimd.dma_start(biasT_t[:], biasT[:]).then_inc(s_gb, 16)
                # Terminal waiter: holds the Pool stream (and thus the
                # Block-end drain+barrier) until outputs are in DRAM.
                gpsimd.wait_ge(s_out, 32)

            @block.tensor
            def _(pe):
                for n in range(NCH):
                    bank = (2 * n) % 6
                    for k in range(NK):
                        if n == 0 and k == 0:
                            pe.wait_ge(s_x1, 16)
                            pe.wait_ge(s_wc[0], 16)
                        if n == 0 and k == 2:
                            pe.wait_ge(s_wc[1], 16)
                        if n == 0 and k == NK // 2:
                            pe.wait_ge(s_x2, 16)
                        if n >= 1 and k == 0:
                            pe.wait_ge(s_wc[n + 1], 16)
                        if n == 3 and k == 0:
                            # banks 0/1 are reused: n0 multiplies must be done
                            pe.wait_ge(s_dvm, 2)
                        for b in range(NB):
                            mm = nc.tensor.matmul(
                                pss[bank + b][:],
                                lhsT=xT_t[:, k, b * P:(b + 1) * P],
                                rhs=wts[n][:, k, :],
                                start=(k == 0),
                                stop=(k == NK - 1),
                            )
                            if k == NK - 1:
                                mm.then_inc(s_pe, 1)
                    if n == 0:
                        # Bias matmuls into dedicated banks 6/7, slotted here
                        # so their input DMAs are long done and the PE stream
                        # never stalls on them.
                        pe.wait_ge(s_gc, 16)
                        pe.wait_ge(s_gb, 16)
                        for b in range(NB):
                            nc.tensor.matmul(
                                pss[6 + b][:, 0:OUTL],
                                lhsT=coefT_t[:, b * P:(b + 1) * P],
                                rhs=biasT_t[:],
                                start=True, stop=True,
                            ).then_inc(s_pe, 1)

            @block.vector
            def _(vector):
                vector.wait_ge(s_gf, 16)
                # s_pe increment order: n0b0=1 n0b1=2 bias0=3 bias1=4
                # n1b0=5 n1b1=6 n2b0=7 n2b1=8 n3b0=9 n3b1=10
                pe_val = {0: (1, 2), 1: (5, 6), 2: (7, 8), 3: (9, 10)}
                g = 0
                for n in range(NCH):
                    bank = (2 * n) % 6
                    for b in range(NB):
                        g += 1
                        vector.wait_ge(s_pe, pe_val[n][b])
                        coef_b = coef_t[:, b, :].rearrange(
                            "p (one r) -> p one r", one=1)
                        tmp = tmps[g % 2]
                        nc.vector.tensor_mul(
                            tmp[:],
                            pss[bank + b][:].rearrange("p (o r) -> p o r", r=RANK),
                            coef_b.to_broadcast((P, OCH, RANK)),
                        ).then_inc(s_dvm, 1)
                        vector.wait_ge(s_dvm, g)
                        nc.vector.tensor_reduce(
                            out_sb[b][:, n * OCH:(n + 1) * OCH],
                            tmp[:],
                            axis=mybir.AxisListType.X,
                            op=mybir.AluOpType.add,
                        ).then_inc(s_red, 1)
                for b in range(NB):
                    vector.wait_ge(s_pe, 3 + b)
                    # all four reduces of this b-chunk (g = b+1, b+3, b+5, b+7)
                    vector.wait_ge(s_red, NB * NCH - NB + b + 1)
                    nc.vector.tensor_add(
                        outf[b][:], out_sb[b][:], pss[6 + b][:, 0:OUTL]
                    ).then_inc(s_dve, 1)

        # After the Block-end drain + all-engine barrier: reset semaphores
        # so a reloaded NEFF can execute again.
        nums = sorted(s.num for s in sems)
        rng = range(nums[0], nums[-1] + 1)
        nc.gpsimd.dma_reset(rng)
        nc.gpsimd.sem_clear(rng)

    nc.compile()
    return nc


def build_nc_tile(dt_name=DT_NAME):
    dt, _ = _DT_MAP[dt_name]
    f32 = mybir.dt.float32
    # Bacc (not raw Bass): its compile() runs generate_event_semaphores,
    # which splits multi-wait sync_info into EventSemaphore prefixes —
    # walrus accepts at most one wait per regular instruction.
    nc = bacc.Bacc("TRN2", target_bir_lowering=False, debug=False)

    xT = nc.declare_dram_parameter("xT", [IN, B], dt, isOutput=False)
    # w2[n, p, k*CH+c] = W2[k*128+p, n*CH+c]: pre-swizzled on host so each
    # SBUF partition's data is one contiguous 8KB run in DRAM (full-rate DMA).
    w2 = nc.declare_dram_parameter("w2", [NCH, P, NK * CH], dt, isOutput=False)
    coef = nc.declare_dram_parameter("coef", [B, RANK], f32, isOutput=False)
    coefT = nc.declare_dram_parameter("coefT", [RANK, B], dt, isOutput=False)
    biasT = nc.declare_dram_parameter("biasT", [RANK, OUTL], dt, isOutput=False)
    out = nc.declare_dram_parameter("out", [B, OUTL], f32, isOutput=True)

    with tile.TileContext(nc) as tc, ExitStack() as ctx:
        cpool = ctx.enter_context(tc.tile_pool(name="const", bufs=1))
        wpool = ctx.enter_context(tc.tile_pool(name="w", bufs=NCH))
        ppool = ctx.enter_context(tc.tile_pool(name="proj", bufs=6, space="PSUM"))
        bpool = ctx.enter_context(tc.tile_pool(name="biasps", bufs=2, space="PSUM"))
        spool = ctx.enter_context(tc.tile_pool(name="stage2", bufs=4))
        opool = ctx.enter_context(tc.tile_pool(name="outp", bufs=2))

        # Weight tiles for every n-chunk (issued first; n=0 split so the
        # first matmuls can start after only 256KB has landed).
        wts = [wpool.tile([P, NK, CH], dt, tag="w", name=f"wt{n}")
               for n in range(NCH)]
        w2v = w2.rearrange("n p (k c) -> n p k c", c=CH)
        nc.sync.dma_start(wts[0][:, 0:2, :], w2v[0][:, 0:2, :])
        # Full inputT, split in halves (first matmuls need only low k).
        xT_t = cpool.tile([P, NK, B], dt, tag="xT")
        xTv = xT.rearrange("(k p) b -> p k b", p=P)
        nc.sync.dma_start(xT_t[:, 0:NK // 2, :], xTv[:, 0:NK // 2, :])
        nc.sync.dma_start(wts[0][:, 2:NK, :], w2v[0][:, 2:NK, :])
        nc.sync.dma_start(xT_t[:, NK // 2:, :], xTv[:, NK // 2:, :])
        for n in range(1, NCH):
            nc.sync.dma_start(wts[n][:], w2[n].rearrange("p (k c) -> p k c", c=CH))
        coef_t = cpool.tile([P, NB, RANK], f32, tag="coef")
        nc.sync.dma_start(coef_t[:], coef.rearrange("(nb p) r -> p nb r", p=P))
        coefT_t = cpool.tile([RANK, B], dt, tag="coefT")
        nc.sync.dma_start(coefT_t[:], coefT[:])
        biasT_t = cpool.tile([RANK, OUTL], dt, tag="biasT")
        nc.sync.dma_start(biasT_t[:], biasT[:])

        # Bias term: out_bias[b,o] = sum_r coef[b,r] * bias[o,r]
        bias_ps = []
        for b in range(NB):
            bp = bpool.tile([P, OUTL], f32, tag="bias")
            nc.tensor.matmul(
                bp[:], lhsT=coefT_t[:, b * P:(b + 1) * P], rhs=biasT_t[:],
                start=True, stop=True,
            )
            bias_ps.append(bp)

        out_sb = [
            opool.tile([P, OUTL], f32, tag="osum", name=f"osum{b}")
            for b in range(NB)
        ]

        for n in range(NCH):
            pss = [
                ppool.tile([P, CH], f32, tag="proj", name=f"proj{n}_{b}")
                for b in range(NB)
            ]
            wt = wts[n]
            for k in range(NK):
                for b in range(NB):
                    nc.tensor.matmul(
                        pss[b][:],
                        lhsT=xT_t[:, k, b * P:(b + 1) * P],
                        rhs=wt[:, k, :],
                        start=(k == 0),
                        stop=(k == NK - 1),
                    )
            # Rank contraction: multiply by per-(b,r) coef, reduce over r.
            for b in range(NB):
                tmp = spool.tile([P, CH], f32, tag="tmp")
                coef_b = coef_t[:, b, :].rearrange("p (one r) -> p one r", one=1)
                nc.vector.tensor_mul(
                    tmp[:].rearrange("p (o r) -> p o r", r=RANK),
                    pss[b][:].rearrange("p (o r) -> p o r", r=RANK),
                    coef_b.to_broadcast((P, OCH, RANK)),
                )
                nc.vector.tensor_reduce(
                    out_sb[b][:, n * OCH:(n + 1) * OCH],
                    tmp[:].rearrange("p (o r) -> p o r", r=RANK),
                    axis=mybir.AxisListType.X,
                    op=mybir.AluOpType.add,
                )

        for b in range(NB):
            outf = opool.tile([P, OUTL], f32, tag="outf")
            nc.vector.tensor_add(outf[:], out_sb[b][:], bias_ps[b][:])
            nc.sync.dma_start(out[b * P:(b + 1) * P, :], outf[:])

    nc.compile()
    return nc


def prepare_in_maps(input, coef, weight, bias, dt_name=DT_NAME):
    _, npdt = _DT_MAP[dt_name]
    xT = np.ascontiguousarray(input.T).astype(npdt)          # (IN, B)
    coefT = np.ascontiguousarray(coef.T).astype(npdt)        # (RANK, B)
    coef32 = np.ascontiguousarray(coef.astype(np.float32))   # (B, RANK)
    in_maps = []
    for c in range(NCORES):
        wsh = weight[c * OUTL:(c + 1) * OUTL]                # (OUTL, IN, RANK)
        # W2[i, o*RANK+r] = wsh[o, i, r]; n-major 512-col chunks; then swizzle
        # (n, i=k*128+p, c) -> (n, p, k, c) so each partition reads one
        # contiguous 8KB run per n-chunk DMA.
        w2 = wsh.transpose(1, 0, 2).reshape(IN, OUTL * RANK)
        w2 = w2.reshape(NK, P, NCH, CH).transpose(2, 1, 0, 3)
        w2 = np.ascontiguousarray(w2.reshape(NCH, P, NK * CH)).astype(npdt)
        biasT = np.ascontiguousarray(
            bias[c * OUTL:(c + 1) * OUTL].T
        ).astype(npdt)                                       # (RANK, OUTL)
        in_maps.append({
            "xT": xT, "w2": w2, "coef": coef32,
            "coefT": coefT, "biasT": biasT,
        })
    return in_maps


_NC_CACHE = {}


def _ensure_ntff_hook():
    """The agent image's antenv lacks axon_hooks; inject it and register
    the ctypes NTFF profile hook so trace=True works under axon."""
    import types
    import antenv
    try:
        from antenv import axon_hooks  # noqa: F401
        return
    except ImportError:
        pass
    mod = types.ModuleType("antenv.axon_hooks")
    _state = {"hook": None}
    mod.set_axon_ntff_profile_hook = lambda h: _state.__setitem__("hook", h)
    mod.get_axon_ntff_profile_hook = lambda: _state["hook"]
    sys.modules["antenv.axon_hooks"] = mod
    antenv.axon_hooks = mod
    try:
        from trn_agent_boot.trn_boot import _ntff_profile_via_ctypes
        mod.set_axon_ntff_profile_hook(
            _ntff_profile_via_ctypes("/opt/axon/libaxon_pjrt.so")
        )
    except Exception:
        pass


def build_nc(dt_name=DT_NAME, impl=None):
    impl = impl or IMPL
    if impl == "raw":
        return build_nc_raw(dt_name)
    return build_nc_tile(dt_name)


def run(inputs, trace=False, dt_name=DT_NAME, impl=None, **kwargs):
    if trace:
        _ensure_ntff_hook()
    impl = impl or IMPL
    key = (dt_name, impl)
    if key not in _NC_CACHE:
        _NC_CACHE[key] = build_nc(dt_name, impl)
    nc = _NC_CACHE[key]
    in_maps = prepare_in_maps(
        np.asarray(inputs["input"], dtype=np.float32),
        np.asarray(inputs["coef"], dtype=np.float32),
        np.asarray(inputs["weight"], dtype=np.float32),
        np.asarray(inputs["bias"], dtype=np.float32),
        dt_name,
    )
    br = run_bass_kernel_spmd(
        nc, in_maps, list(range(NCORES)), trace=trace, **kwargs
    )
    full = np.concatenate(
        [br.results[c]["out"] for c in range(NCORES)], axis=1
    ).astype(np.float32)
    return full, br


def kernel(**inputs):
    full, _ = run(inputs)
    return full


# revision 37
# speedup vs baseline: 1.2812x; 1.0153x over previous
"""MixtureLinear Trainium2 kernel.

Computes, for B=256, IN=1024, OUT=1024, RANK=16:
    out[b,o] = sum_i input[b,i] * sum_r weight[o,i,r] * coef[b,r]
             + sum_r bias[o,r] * coef[b,r]

Strategy (8 NeuronCores, tensor-parallel on OUT):
  - Core c owns OUT rows [128c, 128c+128). It reads only its weight shard
    (1/8 of the 64MB weight tensor), input/coef replicated.
  - Stage 1 (PE): proj[b,(o,r)] = inputT.T @ W2 where W2[i, o*16+r] =
    weight[o,i,r]; K=IN accumulated over 8 psum matmuls per 512-column
    chunk (one psum bank, 32 o's x 16 r's per chunk).
  - Stage 2 (DVE): out[b,o] = sum_r proj[b,(o,r)] * coef[b,r] via a
    broadcast-AP multiply + strided reduce over the innermost rank axis.
  - Bias: one tiny K=16 matmul per b-chunk: coefT.T @ biasT -> psum,
    added in the final DVE add before the output DMA.

Matmul dtype is selectable via MIXL_DT (float16 default; bfloat16 /
float32r / float32 supported). Host pre-casts and pre-transposes shards;
stage-2 and all accumulation stay fp32.
"""

import os
import sys
from contextlib import ExitStack

sys.path.insert(0, "/opt/trn_rl_repo")

import numpy as np
import ml_dtypes

import concourse.bass as bass
import concourse.tile as tile
from concourse import bacc, mybir
from concourse.bass_utils import run_bass_kernel_spmd

B, IN, OUT, RANK = 256, 1024, 1024, 16
NCORES = 8
OUTL = OUT // NCORES        # 128 out rows per core
P = 128                     # partitions
NB = B // P                 # 2 batch chunks
NK = IN // P                # 8 contraction chunks
CH = 512                    # psum chunk: one fp32 bank
NCH = OUTL * RANK // CH     # 4 column chunks per core
OCH = CH // RANK            # 32 o's per chunk

DT_NAME = os.environ.get("MIXL_DT", "float16")
IMPL = os.environ.get("MIXL_IMPL", "raw")

_DT_MAP = {
    "float16": (mybir.dt.float16, np.float16),
    "bfloat16": (mybir.dt.bfloat16, ml_dtypes.bfloat16),
    "float32r": (mybir.dt.float32r, np.float32),
    "float32": (mybir.dt.float32, np.float32),
}


def build_nc_raw(dt_name=DT_NAME):
    """Raw-Bass (manual Block + semaphores) implementation.

    The Tile framework adds ~7us of preamble (serial all-engine barrier
    chain, pool memsets) and ~8us of teardown (drain + EVSEM butterfly +
    sem recycling) — measured 15us of a 38us kernel. This version keeps
    only the work: two HWDGE trigger streams (SP: weights+outputs,
    ACT: inputT), SWDGE for the tiny tensors, PE matmuls, DVE stage-2,
    and a gpsimd tail that waits for the output DMA and resets the
    semaphores so the NEFF stays re-executable.
    """
    dt, _ = _DT_MAP[dt_name]
    f32 = mybir.dt.float32
    nc = bacc.Bacc("TRN2", target_bir_lowering=False, debug=False)

    xT = nc.declare_dram_parameter("xT", [IN, B], dt, isOutput=False)
    w2 = nc.declare_dram_parameter("w2", [NCH, P, NK * CH], dt, isOutput=False)
    coef = nc.declare_dram_parameter("coef", [B, RANK], f32, isOutput=False)
    coefT = nc.declare_dram_parameter("coefT", [RANK, B], dt, isOutput=False)
    biasT = nc.declare_dram_parameter("biasT", [RANK, OUTL], dt, isOutput=False)
    out = nc.declare_dram_parameter("out", [B, OUTL], f32, isOutput=True)

    w2v = w2.rearrange("n p (k c) -> n p k c", c=CH)
    xTv = xT.rearrange("(k p) b -> p k b", p=P)
    coefv = coef.rearrange("(nb p) r -> p nb r", p=P)

    with ExitStack() as ctx:
        sb = lambda shape, d, name: ctx.enter_context(
            nc.sbuf_tensor(name, shape, d))
        xT_t = sb([P, NK, B], dt, "xT_t")
        wts = [sb([P, NK, CH], dt, f"wt{n}") for n in range(NCH)]
        coef_t = sb([P, NB, RANK], f32, "coef_t")
        coefT_t = sb([RANK, B], dt, "coefT_t")
        biasT_t = sb([RANK, OUTL], dt, "biasT_t")
        tmps = [sb([P, OCH, RANK], f32, f"tmp{i}") for i in range(2)]
        out_sb = [sb([P, OUTL], f32, f"osum{b}") for b in range(NB)]
        outf = [sb([P, OUTL], f32, f"outf{b}") for b in range(NB)]
        pss = [ctx.enter_context(nc.psum_tensor(f"ps{g}", [P, CH], f32))
               for g in range(8)]

        # One semaphore per DMA: +16 increments from different transfers
        # interleave (per-SDMA-engine +1s), so aggregate thresholds on a
        # shared sem do not prove any single transfer completed.
        s_wc = [ctx.enter_context(nc.semaphore(f"s_w{i}"))
                for i in range(2 * NCH)]                  # (n, half) chunks
        s_x1 = ctx.enter_context(nc.semaphore("s_x1"))    # inputT low half
        s_x2 = ctx.enter_context(nc.semaphore("s_x2"))    # inputT high half
        s_gc = ctx.enter_context(nc.semaphore("s_gc"))    # coefT
        s_gb = ctx.enter_context(nc.semaphore("s_gb"))    # biasT
        s_gf = ctx.enter_context(nc.semaphore("s_gf"))    # coef (fp32)
        s_pe = ctx.enter_context(nc.semaphore("s_pe"))    # psum groups done
        s_dvm = ctx.enter_context(nc.semaphore("s_dvm"))  # psum mults done
        s_red = ctx.enter_context(nc.semaphore("s_red"))  # reduces done
        s_dve = ctx.enter_context(nc.semaphore("s_dve"))  # outf ready
        s_out = ctx.enter_context(nc.semaphore("s_out"))  # output DMA done
        sems = s_wc + [s_x1, s_x2, s_gc, s_gb, s_gf,
                       s_pe, s_dvm, s_red, s_dve, s_out]

        with nc.Block() as block:

            @block.sync
            def _(sync):
                # All big loads on ONE ring as ~10 chunks with a sliding
                # window of 3 in flight: a single transfer only sustains
                # ~150-200 GB/s, aggregate tops out ~430, and the SDMA
                # engines round-robin across everything queued — so a
                # window keeps both arrival order and full bandwidth.
                H = NK // 2
                xfers = [(xT_t[:, 0:H, :], xTv[:, 0:H, :], s_x1),
                         (wts[0][:, 0:H, :], w2v[0][:, 0:H, :], s_wc[0]),
                         (xT_t[:, H:, :], xTv[:, H:, :], s_x2),
                         (wts[0][:, H:, :], w2v[0][:, H:, :], s_wc[1])]
                for n in range(1, NCH):
                    xfers.append((wts[n][:, 0:H, :], w2v[n][:, 0:H, :],
                                  s_wc[2 * n]))
                    xfers.append((wts[n][:, H:, :], w2v[n][:, H:, :],
                                  s_wc[2 * n + 1]))
                for i, (dst, srcv, sem) in enumerate(xfers):
                    if i >= 3:
                        sync.wait_ge(xfers[i - 3][2], 16)
                    sync.dma_start(dst, srcv).then_inc(sem, 16)

            @block.scalar
            def _(scalar):
                # Output DMAs on the (otherwise idle) ACT ring.
                for b in range(NB):
                    scalar.wait_ge(s_dve, b + 1)
                    scalar.dma_start(out[b * P:(b + 1) * P, :],
                                     outf[b][:]).then_inc(s_out, 16)

            @block.gpsimd
            def _(gpsimd):
                gpsimd.dma_start(coef_t[:], coefv).then_inc(s_gf, 16)
                gpsimd.dma_start(coefT_t[:], coefT[:]).then_inc(s_gc, 16)
                gpsimd.dma_start(biasT_t[:], biasT[:]).then_inc(s_gb, 16)
                # Terminal waiter: holds the Pool stream (and thus the
                # Block-end drain+barrier) until outputs are in DRAM.
                gpsimd.wait_ge(s_out, 32)

            @block.tensor
            def _(pe):
                for n in range(NCH):
                    bank = (2 * n) % 6
                    for k in range(NK):
                        if n == 0 and k == 0:
                            pe.wait_ge(s_x1, 16)
                            pe.wait_ge(s_wc[0], 16)
                        if k == 0 and n >= 1:
                            pe.wait_ge(s_wc[2 * n], 16)
                        if k == NK // 2:
                            pe.wait_ge(s_wc[2 * n + 1], 16)
                            if n == 0:
                                pe.wait_ge(s_x2, 16)
                        if n == 3 and k == 0:
                            # banks 0/1 are reused: n0 multiplies must be done
                            pe.wait_ge(s_dvm, 2)
                        for b in range(NB):
                            # split LDWEIGHTS + non-self-loading matmul: lets
                            # the PE pull the next weight load into the
                            # background buffer while the current MM streams.
                            nc.tensor.ldweights(xT_t[:, k, b * P:(b + 1) * P])
                            mm = nc.tensor.matmul(
                                pss[bank + b][:],
                                lhsT=xT_t[:, k, b * P:(b + 1) * P],
                                rhs=wts[n][:, k, :],
                                start=(k == 0),
                                stop=(k == NK - 1),
                            )
                            mm.ins.ldweights = False
                            if k == NK - 1:
                                mm.then_inc(s_pe, 1)
                    if n == 0:
                        # Bias matmuls into dedicated banks 6/7, slotted here
                        # so their input DMAs are long done and the PE stream
                        # never stalls on them.
                        pe.wait_ge(s_gc, 16)
                        pe.wait_ge(s_gb, 16)
                        for b in range(NB):
                            nc.tensor.matmul(
                                pss[6 + b][:, 0:OUTL],
                                lhsT=coefT_t[:, b * P:(b + 1) * P],
                                rhs=biasT_t[:],
                                start=True, stop=True,
                            ).then_inc(s_pe, 1)

            @block.vector
            def _(vector):
                vector.wait_ge(s_gf, 16)
                # s_pe increment order: n0b0=1 n0b1=2 bias0=3 bias1=4
                # n1b0=5 n1b1=6 n2b0=7 n2b1=8 n3b0=9 n3b1=10
                pe_val = {0: (1, 2), 1: (5, 6), 2: (7, 8), 3: (9, 10)}
                g = 0
                for n in range(NCH):
                    bank = (2 * n) % 6
                    for b in range(NB):
                        g += 1
                        vector.wait_ge(s_pe, pe_val[n][b])
                        coef_b = coef_t[:, b, :].rearrange(
                            "p (one r) -> p one r", one=1)
                        tmp = tmps[g % 2]
                        nc.vector.tensor_mul(
                            tmp[:],
                            pss[bank + b][:].rearrange("p (o r) -> p o r", r=RANK),
                            coef_b.to_broadcast((P, OCH, RANK)),
                        ).then_inc(s_dvm, 1)
                        vector.wait_ge(s_dvm, g)
                        nc.vector.tensor_reduce(
                            out_sb[b][:, n * OCH:(n + 1) * OCH],
                            tmp[:],
                            axis=mybir.AxisListType.X,
                            op=mybir.AluOpType.add,
                        ).then_inc(s_red, 1)
                for b in range(NB):
                    vector.wait_ge(s_pe, 3 + b)
                    # all four reduces of this b-chunk (g = b+1, b+3, b+5, b+7)
                    vector.wait_ge(s_red, NB * NCH - NB + b + 1)
                    nc.vector.tensor_add(
                        outf[b][:], out_sb[b][:], pss[6 + b][:, 0:OUTL]
                    ).then_inc(s_dve, 1)

        # After the Block-end drain + all-engine barrier: reset semaphores
        # so a reloaded NEFF can execute again.
        nums = sorted(s.num for s in sems)
        rng = range(nums[0], nums[-1] + 1)
        nc.gpsimd.dma_reset(rng)
        nc.gpsimd.sem_clear(rng)

    nc.compile()
    return nc


def build_nc_tile(dt_name=DT_NAME):
    dt, _ = _DT_MAP[dt_name]
    f32 = mybir.dt.float32
    # Bacc (not raw Bass): its compile() runs generate_event_semaphores,
    # which splits multi-wait sync_info into EventSemaphore prefixes —
    # walrus accepts at most one wait per regular instruction.
    nc = bacc.Bacc("TRN2", target_bir_lowering=False, debug=False)

    xT = nc.declare_dram_parameter("xT", [IN, B], dt, isOutput=False)
    # w2[n, p, k*CH+c] = W2[k*128+p, n*CH+c]: pre-swizzled on host so each
    # SBUF partition's data is one contiguous 8KB run in DRAM (full-rate DMA).
    w2 = nc.declare_dram_parameter("w2", [NCH, P, NK * CH], dt, isOutput=False)
    coef = nc.declare_dram_parameter("coef", [B, RANK], f32, isOutput=False)
    coefT = nc.declare_dram_parameter("coefT", [RANK, B], dt, isOutput=False)
    biasT = nc.declare_dram_parameter("biasT", [RANK, OUTL], dt, isOutput=False)
    out = nc.declare_dram_parameter("out", [B, OUTL], f32, isOutput=True)

    with tile.TileContext(nc) as tc, ExitStack() as ctx:
        cpool = ctx.enter_context(tc.tile_pool(name="const", bufs=1))
        wpool = ctx.enter_context(tc.tile_pool(name="w", bufs=NCH))
        ppool = ctx.enter_context(tc.tile_pool(name="proj", bufs=6, space="PSUM"))
        bpool = ctx.enter_context(tc.tile_pool(name="biasps", bufs=2, space="PSUM"))
        spool = ctx.enter_context(tc.tile_pool(name="stage2", bufs=4))
        opool = ctx.enter_context(tc.tile_pool(name="outp", bufs=2))

        # Weight tiles for every n-chunk (issued first; n=0 split so the
        # first matmuls can start after only 256KB has landed).
        wts = [wpool.tile([P, NK, CH], dt, tag="w", name=f"wt{n}")
               for n in range(NCH)]
        w2v = w2.rearrange("n p (k c) -> n p k c", c=CH)
        nc.sync.dma_start(wts[0][:, 0:2, :], w2v[0][:, 0:2, :])
        # Full inputT, split in halves (first matmuls need only low k).
        xT_t = cpool.tile([P, NK, B], dt, tag="xT")
        xTv = xT.rearrange("(k p) b -> p k b", p=P)
        nc.sync.dma_start(xT_t[:, 0:NK // 2, :], xTv[:, 0:NK // 2, :])
        nc.sync.dma_start(wts[0][:, 2:NK, :], w2v[0][:, 2:NK, :])
        nc.sync.dma_start(xT_t[:, NK // 2:, :], xTv[:, NK // 2:, :])
        for n in range(1, NCH):
            nc.sync.dma_start(wts[n][:], w2[n].rearrange("p (k c) -> p k c", c=CH))
        coef_t = cpool.tile([P, NB, RANK], f32, tag="coef")
        nc.sync.dma_start(coef_t[:], coef.rearrange("(nb p) r -> p nb r", p=P))
        coefT_t = cpool.tile([RANK, B], dt, tag="coefT")
        nc.sync.dma_start(coefT_t[:], coefT[:])
        biasT_t = cpool.tile([RANK, OUTL], dt, tag="biasT")
        nc.sync.dma_start(biasT_t[:], biasT[:])

        # Bias term: out_bias[b,o] = sum_r coef[b,r] * bias[o,r]
        bias_ps = []
        for b in range(NB):
            bp = bpool.tile([P, OUTL], f32, tag="bias")
            nc.tensor.matmul(
                bp[:], lhsT=coefT_t[:, b * P:(b + 1) * P], rhs=biasT_t[:],
                start=True, stop=True,
            )
            bias_ps.append(bp)

        out_sb = [
            opool.tile([P, OUTL], f32, tag="osum", name=f"osum{b}")
            for b in range(NB)
        ]

        for n in range(NCH):
            pss = [
                ppool.tile([P, CH], f32, tag="proj", name=f"proj{n}_{b}")
                for b in range(NB)
            ]
            wt = wts[n]
            for k in range(NK):
                for b in range(NB):
                    nc.tensor.matmul(
                        pss[b][:],
                        lhsT=xT_t[:, k, b * P:(b + 1) * P],
                        rhs=wt[:, k, :],
                        start=(k == 0),
                        stop=(k == NK - 1),
                    )
            # Rank contraction: multiply by per-(b,r) coef, reduce over r.
            for b in range(NB):
                tmp = spool.tile([P, CH], f32, tag="tmp")
                coef_b = coef_t[:, b, :].rearrange("p (one r) -> p one r", one=1)
                nc.vector.tensor_mul(
                    tmp[:].rearrange("p (o r) -> p o r", r=RANK),
                    pss[b][:].rearrange("p (o r) -> p o r", r=RANK),
                    coef_b.to_broadcast((P, OCH, RANK)),
                )
                nc.vector.tensor_reduce(
                    out_sb[b][:, n * OCH:(n + 1) * OCH],
                    tmp[:].rearrange("p (o r) -> p o r", r=RANK),
                    axis=mybir.AxisListType.X,
                    op=mybir.AluOpType.add,
                )

        for b in range(NB):
            outf = opool.tile([P, OUTL], f32, tag="outf")
            nc.vector.tensor_add(outf[:], out_sb[b][:], bias_ps[b][:])
            nc.sync.dma_start(out[b * P:(b + 1) * P, :], outf[:])

    nc.compile()
    return nc


def prepare_in_maps(input, coef, weight, bias, dt_name=DT_NAME):
    _, npdt = _DT_MAP[dt_name]
    xT = np.ascontiguousarray(input.T).astype(npdt)          # (IN, B)
    coefT = np.ascontiguousarray(coef.T).astype(npdt)        # (RANK, B)
    coef32 = np.ascontiguousarray(coef.astype(np.float32))   # (B, RANK)
    in_maps = []
    for c in range(NCORES):
        wsh = weight[c * OUTL:(c + 1) * OUTL]                # (OUTL, IN, RANK)
        # W2[i, o*RANK+r] = wsh[o, i, r]; n-major 512-col chunks; then swizzle
        # (n, i=k*128+p, c) -> (n, p, k, c) so each partition reads one
        # contiguous 8KB run per n-chunk DMA.
        w2 = wsh.transpose(1, 0, 2).reshape(IN, OUTL * RANK)
        w2 = w2.reshape(NK, P, NCH, CH).transpose(2, 1, 0, 3)
        w2 = np.ascontiguousarray(w2.reshape(NCH, P, NK * CH)).astype(npdt)
        biasT = np.ascontiguousarray(
            bias[c * OUTL:(c + 1) * OUTL].T
        ).astype(npdt)                                       # (RANK, OUTL)
        in_maps.append({
            "xT": xT, "w2": w2, "coef": coef32,
            "coefT": coefT, "biasT": biasT,
        })
    return in_maps


_NC_CACHE = {}


def _ensure_ntff_hook():
    """The agent image's antenv lacks axon_hooks; inject it and register
    the ctypes NTFF profile hook so trace=True works under axon."""
    import types
    import antenv
    try:
        from antenv import axon_hooks  # noqa: F401
        return
    except ImportError:
        pass
    mod = types.ModuleType("antenv.axon_hooks")
    _state = {"hook": None}
    mod.set_axon_ntff_profile_hook = lambda h: _state.__setitem__("hook", h)
    mod.get_axon_ntff_profile_hook = lambda: _state["hook"]
    sys.modules["antenv.axon_hooks"] = mod
    antenv.axon_hooks = mod
    try:
        from trn_agent_boot.trn_boot import _ntff_profile_via_ctypes
        mod.set_axon_ntff_profile_hook(
            _ntff_profile_via_ctypes("/opt/axon/libaxon_pjrt.so")
        )
    except Exception:
        pass


def build_nc(dt_name=DT_NAME, impl=None):
    impl = impl or IMPL
    if impl == "raw":
        return build_nc_raw(dt_name)
    return build_nc_tile(dt_name)


def run(inputs, trace=False, dt_name=DT_NAME, impl=None, **kwargs):
    if trace:
        _ensure_ntff_hook()
    impl = impl or IMPL
    key = (dt_name, impl)
    if key not in _NC_CACHE:
        _NC_CACHE[key] = build_nc(dt_name, impl)
    nc = _NC_CACHE[key]
    in_maps = prepare_in_maps(
        np.asarray(inputs["input"], dtype=np.float32),
        np.asarray(inputs["coef"], dtype=np.float32),
        np.asarray(inputs["weight"], dtype=np.float32),
        np.asarray(inputs["bias"], dtype=np.float32),
        dt_name,
    )
    br = run_bass_kernel_spmd(
        nc, in_maps, list(range(NCORES)), trace=trace, **kwargs
    )
    full = np.concatenate(
        [br.results[c]["out"] for c in range(NCORES)], axis=1
    ).astype(np.float32)
    return full, br


def kernel(**inputs):
    full, _ = run(inputs)
    return full
